# revision 1
# baseline (speedup 1.0000x reference)
"""Trainium2 Bass kernel for a spatial self-attention block.

reference computation (B=4, H=W=64, C=512, N=H*W=4096):
    h = group_norm(x, gamma, beta, 32 groups)
    q,k,v = h@wq+bq, h@wk+bk, h@wv+bv
    scores = (q @ k^T) / sqrt(C); attn = softmax(scores, -1)
    out = (attn @ v) @ wo + bo + x

Sharding: 8 cores = (batch b in 0..3) x (query-half in 0..1). Each core
computes group-norm stats + K/V for its full batch element (duplicated
across the pair) and attention outputs for its own 2048 query rows.
The host permutes each core's batch rows so its own queries are rows
0:2048 — attention is permutation-invariant over keys, so one uniform
SPMD program works for all cores.

Group norm is folded into the QKV projections: h = x*s + t with
per-channel s,t from the batch stats, so q = x @ (diag(s) wq) + (t@wq+bq).

Precision: group-norm statistics and the x-transposes run in
float32r (TF32-like); K/Q/V and the softmax exponentials are stored as
fp16 (score range is ~[-7, 7] by construction, so exp fits comfortably),
which enables fast-weight-load on the PE (216 ns/matmul) and halves
SBUF so V stays resident on-chip. The softmax denominator path and the
output projection stay in fp32r/fp32.

Attention uses a transposed-scores layout sT[j_key, i_query]; attn@V is
computed transposed (avT[c] += v[:,c-slice].T @ exp) so the result is
channel-major and feeds the O-projection with no transposes. The
1/denominator scale is applied after the O-projection (query index is
then the partition dim) and bo + x arrive pre-summed from the host
(xbo). The V bias is folded in as an outer product bv (x) denom added
to the unnormalized accumulator (softmax rows sum to denom).

Packed host constants tensor `consts` [128, 900] (fp32 bits):
  cols 0:128   identity matrix (PE transposes)
  col  128     ones column  [128,1]
  cols 129:257 ones row     [1,128] (partition 0)
  cols 772:900 all-ones     [128,128]
"""

import sys

import numpy as np

if "/opt/trn_rl_repo" not in sys.path:
    sys.path.insert(0, "/opt/trn_rl_repo")

import concourse.mybir as mybir
import concourse.tile as tile
from concourse import bacc
from concourse.bass_utils import run_bass_kernel_spmd

F32 = mybir.dt.float32
F32R = mybir.dt.float32r
F16 = mybir.dt.float16

B, N, C = 4, 4096, 512
HALF = N // 2          # own query rows per core
G = 32                 # groups
GS = C // G            # channels per group
P = 128                # partitions
CO = C // P            # channel subtiles (4)
N_CORES = 8
EPS = 1e-6
SM_SCALE = 1.0 / float(np.sqrt(C))
I_CHUNK = 512          # query-chunk per attention sweep
N_CHUNKS = HALF // I_CHUNK   # 4
JT = N // P            # 32 key tiles
NT = N // P            # 32 row tiles per batch
HT = HALF // P         # 16 row tiles per half
AF = mybir.ActivationFunctionType


def _f(ap):
    return ap.bitcast(F32)


def build_nc():
    nc = bacc.Bacc("TRN2", target_bir_lowering=False, num_devices=N_CORES)

    xb = nc.dram_tensor("xb", [N, C], F32R, kind="ExternalInput")
    wq_d = nc.dram_tensor("wq", [C, C], F32R, kind="ExternalInput")
    wk_d = nc.dram_tensor("wk", [C, C], F32R, kind="ExternalInput")
    wv_d = nc.dram_tensor("wv", [C, C], F32R, kind="ExternalInput")
    wo_d = nc.dram_tensor("wo", [C, C], F32R, kind="ExternalInput")
    bq_d = nc.dram_tensor("bq", [C], F32R, kind="ExternalInput")
    bk_d = nc.dram_tensor("bk", [C], F32R, kind="ExternalInput")
    bv_d = nc.dram_tensor("bv", [C], F32R, kind="ExternalInput")
    gamma_d = nc.dram_tensor("gn_gamma", [C], F32R, kind="ExternalInput")
    beta_d = nc.dram_tensor("gn_beta", [C], F32R, kind="ExternalInput")
    consts_d = nc.dram_tensor("consts", [P, 900], F32R, kind="ExternalInput")
    xbo_d = nc.dram_tensor("xbo", [HALF, C], F32R, kind="ExternalInput")
    out_d = nc.dram_tensor("out", [HALF, C], F32, kind="ExternalOutput")

    xb_t = xb[:].rearrange("(t p) c -> t p c", p=P)       # 32 x [128, 512]
    xbo_t = xbo_d[:].rearrange("(t p) c -> t p c", p=P)   # 16 x [128, 512]
    out_t = out_d[:].rearrange("(t p) c -> t p c", p=P)   # 16 x [128, 512]

    with tile.TileContext(nc) as tc:
        with (
            tc.tile_pool(name="persist", bufs=1) as persist,
            tc.tile_pool(name="cpool", bufs=1) as cpool,
            tc.tile_pool(name="keep", bufs=1) as keep,
            tc.tile_pool(name="xstage", bufs=8) as xstage,
        ):
            kT = persist.tile([P, CO, N], F16, tag="kT")
            qT = persist.tile([P, CO, HALF], F16, tag="qT")
            v_sb = persist.tile([P, NT, C], F16, tag="v_sb")

            consts = cpool.tile([P, 900], F32R, tag="consts")
            nc.sync.dma_start(consts[:], consts_d[:])
            ident = consts[:, 0:P]
            ones_col = consts[:, P:P + 1]
            ones_row = consts[0:1, 129:257]
            allones = consts[:, 772:900]

            parts = keep.tile([P, 4 * CO], F32R, tag="parts")
            s_part = parts[:, 0:CO]
            t_part = parts[:, CO:2 * CO]
            bqp = parts[:, 2 * CO:3 * CO]
            bkp = parts[:, 3 * CO:4 * CO]
            bv_eff = keep.tile([1, C], F32R, tag="bv_eff")

            with (
                tc.tile_pool(name="w32p", bufs=1) as w32p,
                tc.tile_pool(name="w16p", bufs=1) as w16p,
                tc.tile_pool(name="stats_ps", bufs=1, space="PSUM") as stats_ps,
                tc.tile_pool(name="sqpool", bufs=3) as sqpool,
                tc.tile_pool(name="prows", bufs=1) as prows,
                tc.tile_pool(name="xT_pool", bufs=1) as xT_pool,
                tc.tile_pool(name="xpose_ps", bufs=2, space="PSUM") as xpose_ps,
            ):
                # packed small rows: inputs and worksheets
                irows = prows.tile([1, 5 * C], F32R, tag="irows")
                gamma_row = irows[:, 0 * C:1 * C]
                beta_row = irows[:, 1 * C:2 * C]
                bq_row = irows[:, 2 * C:3 * C]
                bk_row = irows[:, 3 * C:4 * C]
                bv_row = irows[:, 4 * C:5 * C]
                wrows = prows.tile([1, 4 * C], F32, tag="wrows")
                sum_row = wrows[:, 0 * C:1 * C]
                sq_row = wrows[:, 1 * C:2 * C]
                s_row = wrows[:, 2 * C:3 * C].bitcast(F32R)
                t_row = wrows[:, 3 * C:4 * C].bitcast(F32R)
                berows = prows.tile([1, 2 * C], F32R, tag="berows")
                grows = prows.tile([1, 3 * G], F32, tag="grows")
                g_mean = grows[:, 0:G]
                g_var = grows[:, G:2 * G]
                g_tmp = grows[:, 2 * G:3 * G]

                # ---- single x pass: stats matmuls + transpose into fp16 xT ----
                s_ps = stats_ps.tile([P, C], F32, tag="S")
                q_ps = stats_ps.tile([P, C], F32, tag="Q")
                xT = xT_pool.tile([P, CO, N], F16, tag="xT", name="xT")
                for t in range(NT):
                    xt = xstage.tile([P, C], F32R, tag="xt")
                    if t % 2 == 0:
                        nc.sync.dma_start(xt[:], xb_t[t])
                    else:
                        nc.gpsimd.dma_start(xt[:], xb_t[t])
                    nc.tensor.matmul(s_ps[:], (allones), (xt[:]),
                                     start=(t == 0), stop=(t == NT - 1))
                    sq = sqpool.tile([P, C], F32R, tag="sq")
                    nc.scalar.activation(sq[:], xt[:], AF.Square)
                    nc.tensor.matmul(q_ps[:], (allones), (sq[:]),
                                     start=(t == 0), stop=(t == NT - 1))
                    pps = xpose_ps.tile([P, C], F32R, tag="xpose", name="pps")
                    for o in range(CO):
                        nc.tensor.matmul(pps[:, o * P:(o + 1) * P],
                                         xt[:, o * P:(o + 1) * P],
                                         ident, is_transpose=True,
                                         start=(o == 0), stop=(o == CO - 1))
                    nc.vector.tensor_copy(
                        xT[:, :, t * P:(t + 1) * P],
                        pps[:].rearrange("p (o i) -> p o i", o=CO))

                ws32 = {}
                for name, src_d in (("wq", wq_d), ("wk", wk_d), ("wv", wv_d)):
                    w = w32p.tile([P, CO, C], F32R, tag=name, name=name)
                    for o in range(CO):
                        nc.sync.dma_start(w[:, o, :], src_d[o * P:(o + 1) * P, :])
                    ws32[name] = w

                for i, src_d in enumerate((gamma_d, beta_d, bq_d, bk_d,
                                           bv_d)):
                    nc.sync.dma_start(irows[:, i * C:(i + 1) * C],
                                      src_d[:][None, :])

                # ---- group stats -> per-channel scale/shift ----
                nc.vector.tensor_copy(sum_row, s_ps[0:1, :])
                nc.vector.tensor_copy(sq_row, q_ps[0:1, :])
                inv_cnt = 1.0 / (N * GS)
                nc.vector.reduce_sum(g_mean,
                                     sum_row.rearrange("p (g e) -> p g e", e=GS),
                                     axis=mybir.AxisListType.X)
                nc.vector.tensor_scalar_mul(g_mean, g_mean, inv_cnt)
                nc.vector.reduce_sum(g_var,
                                     sq_row.rearrange("p (g e) -> p g e", e=GS),
                                     axis=mybir.AxisListType.X)
                nc.vector.tensor_scalar_mul(g_var, g_var, inv_cnt)
                nc.vector.tensor_mul(g_tmp, g_mean, g_mean)
                nc.vector.tensor_sub(g_var, g_var, g_tmp)
                nc.vector.tensor_scalar_add(g_var, g_var, EPS)
                nc.scalar.activation(g_tmp, g_var, AF.Sqrt)
                nc.vector.reciprocal(g_tmp, g_tmp)  # rstd per group

                sv = s_row.rearrange("p (g e) -> p g e", e=GS)
                tv = t_row.rearrange("p (g e) -> p g e", e=GS)
                gv = gamma_row.rearrange("p (g e) -> p g e", e=GS)
                nc.vector.tensor_tensor(
                    sv, gv, g_tmp[:, :, None].to_broadcast((1, G, GS)),
                    mybir.AluOpType.mult)
                nc.vector.tensor_tensor(
                    tv, sv, g_mean[:, :, None].to_broadcast((1, G, GS)),
                    mybir.AluOpType.mult)
                nc.vector.tensor_sub(t_row, beta_row, t_row)

                with tc.tile_pool(name="pize_ps", bufs=1, space="PSUM") as pize_ps:
                    for vec_row, dst in ((s_row, s_part), (t_row, t_part)):
                        pp = pize_ps.tile([P, CO], F32, tag="pize", name="pp")
                        for o in range(CO):
                            nc.tensor.matmul(pp[:, o:o + 1],
                                             _f(vec_row[0:1, o * P:(o + 1) * P]),
                                             _f(ones_row[0:1, 0:1]),
                                             start=(o == 0), stop=(o == CO - 1))
                        nc.vector.tensor_copy(dst, pp[:])

                    # effective biases b' = t @ W + b (unfolded fp32r weights)
                    beff = {"wq": berows[:, 0:C], "wk": berows[:, C:2 * C],
                            "wv": bv_eff[:]}
                    for name, brow in (("wq", bq_row), ("wk", bk_row),
                                       ("wv", bv_row)):
                        bps = stats_ps.tile([1, C], F32, tag="S", name="bps")
                        for o in range(CO):
                            nc.tensor.matmul(bps[:], (t_part[:, o:o + 1]),
                                             (ws32[name][:, o, :]),
                                             start=(o == 0), stop=(o == CO - 1))
                        nc.vector.tensor_add(beff[name], bps[:], brow)

                    for vec_row, dst in ((beff["wq"], bqp), (beff["wk"], bkp)):
                        pp = pize_ps.tile([P, CO], F32, tag="pize", name="pp")
                        for o in range(CO):
                            nc.tensor.matmul(pp[:, o:o + 1],
                                             _f(vec_row[0:1, o * P:(o + 1) * P]),
                                             _f(ones_row[0:1, 0:1]),
                                             start=(o == 0), stop=(o == CO - 1))
                        nc.vector.tensor_copy(dst, pp[:])

                # fold group-norm scale into fp16 copies of wq/wk/wv
                ws16 = {}
                for name in ("wq", "wk", "wv"):
                    w16 = w16p.tile([P, CO, C], F16, tag=name, name=f"{name}16")
                    for o in range(CO):
                        nc.vector.tensor_scalar_mul(w16[:, o, :],
                                                    ws32[name][:, o, :],
                                                    _f(s_part[:, o:o + 1]))
                    ws16[name] = w16

                # ---- projections (fp16): kT, qT, v resident in SBUF ----
                with tc.tile_pool(name="proj_ps", bufs=1, space="PSUM") as proj_ps:
                    for o in range(CO):
                        for jcb in range(2):   # blocks of 4 x 512 columns
                            kpss = [proj_ps.tile([P, 512], F32, tag=f"proj{jc}",
                                                 name=f"kps{jc}")
                                    for jc in range(4)]
                            for ci in range(CO):
                                for jc in range(4):
                                    col = (jcb * 4 + jc) * 512
                                    nc.tensor.matmul(
                                        kpss[jc][:],
                                        (ws16["wk"][:, ci, o * P:(o + 1) * P]),
                                        (xT[:, ci, col:col + 512]),
                                        start=(ci == 0), stop=(ci == CO - 1))
                            for jc in range(4):
                                col = (jcb * 4 + jc) * 512
                                nc.scalar.activation(
                                    kT[:, o, col:col + 512], kpss[jc][:],
                                    AF.Identity, bias=_f(bkp[:, o:o + 1]))

                    for o in range(CO):
                        qpss = [proj_ps.tile([P, 512], F32, tag=f"proj{jc}",
                                             name=f"qps{jc}")
                                for jc in range(4)]
                        for ci in range(CO):
                            for jc in range(4):
                                nc.tensor.matmul(
                                    qpss[jc][:],
                                    (ws16["wq"][:, ci, o * P:(o + 1) * P]),
                                    (xT[:, ci, jc * 512:(jc + 1) * 512]),
                                    start=(ci == 0), stop=(ci == CO - 1))
                        for jc in range(4):
                            nc.scalar.activation(
                                qT[:, o, jc * 512:(jc + 1) * 512], qpss[jc][:],
                                AF.Identity, bias=_f(bqp[:, o:o + 1]))

                    # v rows (bias folded in later via denom outer-product)
                    for t16 in range(NT):
                        vps = proj_ps.tile([P, C], F32, tag=f"proj{t16 % 4}",
                                           name="vps")
                        for ci in range(CO):
                            nc.tensor.matmul(vps[:],
                                             (xT[:, ci, t16 * P:(t16 + 1) * P]),
                                             (ws16["wv"][:, ci, :]),
                                             start=(ci == 0), stop=(ci == CO - 1))
                        if t16 % 2 == 0:
                            nc.vector.tensor_copy(v_sb[:, t16, :], vps[:])
                        else:
                            nc.scalar.activation(v_sb[:, t16, :], vps[:], AF.Copy)

            # ---- attention + output projection + residual ----
            with (
                tc.tile_pool(name="wop", bufs=1) as wop,
                tc.tile_pool(name="sT_ps", bufs=2, space="PSUM") as sT_ps,
                tc.tile_pool(name="av_ps", bufs=1, space="PSUM") as av_ps,
                tc.tile_pool(name="sh_ps", bufs=2, space="PSUM") as sh_ps,
                tc.tile_pool(name="expp", bufs=4) as expp,
                tc.tile_pool(name="accp", bufs=2) as accp,
                tc.tile_pool(name="aoT", bufs=2) as aoTp,
                tc.tile_pool(name="ostage", bufs=2) as ostage,
                tc.tile_pool(name="xres", bufs=2) as xres,
                tc.tile_pool(name="drow", bufs=2) as drow,
            ):
                wo_sb = wop.tile([P, CO, C], F32R, tag="wo", name="wo_sb")
                for o in range(CO):
                    nc.sync.dma_start(wo_sb[:, o, :], wo_d[o * P:(o + 1) * P, :])

                for chunk in range(N_CHUNKS):
                    i0 = chunk * I_CHUNK
                    avs = [av_ps.tile([P, I_CHUNK], F32, tag=f"av{i}",
                                      name=f"av{i}")
                           for i in range(CO)]
                    acc_a = accp.tile([P, I_CHUNK], F32, tag="acc_a")
                    acc_b = accp.tile([P, I_CHUNK], F32, tag="acc_b")
                    for j in range(JT):
                        sps = sT_ps.tile([P, I_CHUNK], F32, tag="sT", name="sps")
                        for ci in range(CO):
                            nc.tensor.matmul(
                                sps[:],
                                (kT[:, ci, j * P:(j + 1) * P]),
                                (qT[:, ci, i0:i0 + I_CHUNK]),
                                start=(ci == 0), stop=(ci == CO - 1))
                        ex = expp.tile([P, I_CHUNK], F16, tag="ex")
                        nc.scalar.activation(ex[:], sps[:], AF.Exp,
                                             scale=SM_SCALE)
                        for cs in range(CO):
                            nc.tensor.matmul(avs[cs][:],
                                             (v_sb[:, j, cs * P:(cs + 1) * P]),
                                             (ex[:]),
                                             start=(j == 0), stop=False)
                        # denominator partials: alternate DVE / GpSimd
                        if j == 0:
                            nc.vector.tensor_copy(acc_a[:], ex[:])
                        elif j == 1:
                            nc.gpsimd.tensor_copy(acc_b[:], ex[:])
                        elif j % 2 == 0:
                            nc.vector.tensor_add(acc_a[:], acc_a[:], ex[:])
                        else:
                            nc.gpsimd.tensor_add(acc_b[:], acc_b[:], ex[:])

                    nc.vector.tensor_add(acc_a[:], acc_a[:], acc_b[:])
                    dps = sh_ps.tile([1, I_CHUNK], F32, tag="sh", name="dps")
                    nc.tensor.matmul(dps[:], _f(ones_col), _f(acc_a[:]),
                                     start=True, stop=True)
                    d_row = drow.tile([1, I_CHUNK], F32R, tag="d_row")
                    nc.vector.tensor_copy(d_row[:], dps[:])
                    # V-bias: avT += bv (x) denom (unnormalized rows sum to denom)
                    for cs in range(CO):
                        nc.tensor.matmul(avs[cs][:],
                                         (bv_eff[0:1, cs * P:(cs + 1) * P]),
                                         (d_row[:]),
                                         start=False, stop=True)
                    dp = sh_ps.tile([P, 4], F32, tag="sh", name="dp")
                    for o in range(4):
                        nc.tensor.matmul(dp[:, o:o + 1],
                                         _f(d_row[0:1, o * P:(o + 1) * P]),
                                         _f(ones_row[0:1, 0:1]),
                                         start=(o == 0), stop=(o == 3))
                    d_inv = drow.tile([P, 4], F32, tag="d_inv")
                    nc.vector.reciprocal(d_inv[:], dp[:])

                    aoT = aoTp.tile([P, CO, I_CHUNK], F32R, tag="aoT")
                    for cs in range(CO):
                        if cs % 2 == 0:
                            nc.vector.tensor_copy(aoT[:, cs, :], avs[cs][:])
                        else:
                            nc.scalar.activation(aoT[:, cs, :], avs[cs][:],
                                                 AF.Copy)

                    for it in range(4):
                        ops = sh_ps.tile([P, C], F32, tag="sh", name="ops")
                        for ci in range(CO):
                            nc.tensor.matmul(ops[:],
                                             (aoT[:, ci, it * P:(it + 1) * P]),
                                             (wo_sb[:, ci, :]),
                                             start=(ci == 0), stop=(ci == CO - 1))
                        xr = xres.tile([P, C], F32R, tag="xr")
                        nc.sync.dma_start(xr[:], xbo_t[chunk * 4 + it])
                        ot = ostage.tile([P, C], F32, tag="ot")
                        nc.vector.scalar_tensor_tensor(
                            ot[:], ops[:], _f(d_inv[:, it:it + 1]), xr[:],
                            mybir.AluOpType.mult, mybir.AluOpType.add)
                        nc.sync.dma_start(out_t[chunk * 4 + it], ot[:])

    nc.compile()
    return nc


_NC = None


def _get_nc():
    global _NC
    if _NC is None:
        _NC = build_nc()
    return _NC


def make_consts():
    consts = np.zeros((P, 900), np.float32)
    consts[:, 0:P] = np.eye(P, dtype=np.float32)
    consts[:, P] = 1.0
    consts[0, 129:257] = 1.0
    consts[:, 772:900] = 1.0
    return consts


def make_in_maps(x, gn_gamma, gn_beta, wq, bq, wk, bk, wv, bv, wo, bo):
    x4 = np.ascontiguousarray(np.asarray(x, np.float32).reshape(B, N, C))
    consts = make_consts()
    bo_f = np.asarray(bo, np.float32)
    common = dict(
        wq=np.asarray(wq, np.float32), wk=np.asarray(wk, np.float32),
        wv=np.asarray(wv, np.float32), wo=np.asarray(wo, np.float32),
        bq=np.asarray(bq, np.float32), bk=np.asarray(bk, np.float32),
        bv=np.asarray(bv, np.float32),
        gn_gamma=np.asarray(gn_gamma, np.float32),
        gn_beta=np.asarray(gn_beta, np.float32),
        consts=consts,
    )
    in_maps = []
    for c in range(N_CORES):
        b, h = c // 2, c % 2
        own = x4[b, h * HALF:(h + 1) * HALF]
        other = x4[b, (1 - h) * HALF:(2 - h) * HALF]
        xb_ = np.ascontiguousarray(np.concatenate([own, other], axis=0))
        xbo = np.ascontiguousarray(own + bo_f)
        in_maps.append(dict(xb=xb_, xbo=xbo, **common))
    return in_maps


def assemble(results):
    out = np.empty((B, N, C), np.float32)
    for c in range(N_CORES):
        b, h = c // 2, c % 2
        out[b, h * HALF:(h + 1) * HALF] = results[c]["out"]
    return out.reshape(B, 64, 64, C)


def kernel(**inputs):
    nc = _get_nc()
    in_maps = make_in_maps(**inputs)
    res = run_bass_kernel_spmd(nc, in_maps, list(range(N_CORES)))
    return assemble(res.results)



# revision 7
# speedup vs baseline: 1.5403x; 1.5403x over previous
"""Trainium2 Bass kernel for a spatial self-attention block (fp8 DoubleRow).

reference computation (B=4, H=W=64, C=512, N=H*W=4096):
    h = group_norm(x, gamma, beta, 32 groups)
    q,k,v = h@wq+bq, h@wk+bk, h@wv+bv
    scores = (q @ k^T) / sqrt(C); attn = softmax(scores, -1)
    out = (attn @ v) @ wo + bo + x

Sharding: 8 cores = (batch b in 0..3) x (query-half in 0..1). Each core
computes group-norm stats + K/V for its full batch element (duplicated
across the pair) and attention outputs for its own 2048 query rows.
The host permutes each core's batch rows so its own queries are rows
0:2048 — attention is permutation-invariant over keys, so one uniform
SPMD program works for all cores.

Precision strategy (rel-err budget 2e-2; measured ~8e-3 in numpy sim):
  - x arrives channel-major (host-transposed) pre-cast to fp8 e4m3.
  - group-norm stats from the fp8 x: channel sums on DVE, channel
    sum-of-squares on ScalarE (Square+accum_out) and DVE
    (tensor_tensor_reduce); a tiny PE transpose moves the per-channel
    [128,8] stat block into rows for the group reduction chain.
  - group norm is folded into the projections: h = x*s + t, so
    q = x @ (diag(s) wq) + (t@wq + bq). Weights arrive fp16 scaled by
    16 (keeps the fp8 quantization of s*w in the normal range); the
    fold multiplies by the per-channel s and casts to fp8. The 1/16
    is applied when the projection PSUM is written out.
  - all the big matmuls (QKV projections, scores, attn@V) run fp8
    e4m3 with perf_mode=DoubleRow: 3-D APs [128, 2, m] contract 256
    channels/keys per instruction at 2 rows/cycle.
  - softmax: exp(s/sqrt(C) - 3) computed on ScalarE straight into fp8
    (max score ~6.7 so exp <= 42, inside e4m3's 240 max normal). The
    denominator sums the same fp8 values, so softmax stays exactly
    normalized; V's bias is folded in as bv (x) denom added to the
    unnormalized accumulator.
  - the output projection runs fp16 (the unnormalized attn@V values
    can exceed fp8 range); 1/denominator is applied after it, where
    the query index is the partition dim. bo + x arrive pre-summed
    from the host (xbo).

Packed host constants tensor `consts` [128, 900] (fp32 bits):
  cols 0:128   identity matrix (PE transposes)
  col  128     ones column  [128,1]
  cols 129:257 ones row     [1,128] (partition 0)
"""

import sys

import numpy as np
import ml_dtypes

if "/opt/trn_rl_repo" not in sys.path:
    sys.path.insert(0, "/opt/trn_rl_repo")

import concourse.mybir as mybir
import concourse.tile as tile
from concourse import bacc
from concourse.bass_utils import run_bass_kernel_spmd

F32 = mybir.dt.float32
F32R = mybir.dt.float32r
F16 = mybir.dt.float16
F8 = mybir.dt.float8e4
DR = mybir.MatmulPerfMode.DoubleRow

B, N, C = 4, 4096, 512
HALF = N // 2          # own query rows per core
G = 32                 # groups
GS = C // G            # channels per group
P = 128                # partitions
CO = C // P            # channel subtiles (4)
CPAIR = CO // 2        # channel subtile pairs for DoubleRow (2)
N_CORES = 8
EPS = 1e-6
SM_SCALE = 1.0 / float(np.sqrt(C))
SHIFT = 3.0            # exp(score - SHIFT); max score ~6.7 -> exp <= 42
WSCALE = 16.0          # host scales w by 16 so s*w lands in fp8 normals
I_CHUNK = 512          # query-chunk per attention sweep
N_CHUNKS = HALF // I_CHUNK   # 4
JT = N // P            # 32 key tiles
JPAIR = JT // 2        # 16 key-tile pairs (DoubleRow contracts 256 keys)
NT = N // P            # 32 row tiles per batch
AF = mybir.ActivationFunctionType
ALU = mybir.AluOpType


def _f(ap):
    return ap.bitcast(F32)


def build_nc():
    nc = bacc.Bacc("TRN2", target_bir_lowering=False, num_devices=N_CORES)

    xT8_d = nc.dram_tensor("xT8", [C, N], F8, kind="ExternalInput")
    wq_d = nc.dram_tensor("wq16", [C, C], F16, kind="ExternalInput")
    wk_d = nc.dram_tensor("wk16", [C, C], F16, kind="ExternalInput")
    wv_d = nc.dram_tensor("wv16", [C, C], F16, kind="ExternalInput")
    wo_d = nc.dram_tensor("wo16", [C, C], F16, kind="ExternalInput")
    bq_d = nc.dram_tensor("bq", [C], F32R, kind="ExternalInput")
    bk_d = nc.dram_tensor("bk", [C], F32R, kind="ExternalInput")
    bv_d = nc.dram_tensor("bv", [C], F32R, kind="ExternalInput")
    gamma_d = nc.dram_tensor("gn_gamma", [C], F32R, kind="ExternalInput")
    beta_d = nc.dram_tensor("gn_beta", [C], F32R, kind="ExternalInput")
    consts_d = nc.dram_tensor("consts", [P, 900], F32R, kind="ExternalInput")
    xbo_d = nc.dram_tensor("xbo", [HALF, C], F32R, kind="ExternalInput")
    out_d = nc.dram_tensor("out", [HALF, C], F32, kind="ExternalOutput")

    xbo_t = xbo_d[:].rearrange("(t p) c -> t p c", p=P)   # 16 x [128, 512]
    out_t = out_d[:].rearrange("(t p) c -> t p c", p=P)   # 16 x [128, 512]

    with tile.TileContext(nc) as tc:
        with (
            tc.tile_pool(name="persist", bufs=1) as persist,
            tc.tile_pool(name="cpool", bufs=1) as cpool,
            tc.tile_pool(name="keep", bufs=1) as keep,
        ):
            kT8 = persist.tile([P, CO, N], F8, tag="kT8")
            qT8 = persist.tile([P, CO, HALF], F8, tag="qT8")
            v8 = persist.tile([P, NT, C], F8, tag="v8")

            consts = cpool.tile([P, 900], F32R, tag="consts")
            nc.sync.dma_start(consts[:], consts_d[:])
            ident = consts[:, 0:P]
            ones_col = consts[:, P:P + 1]
            ones_row = consts[0:1, 129:257]
            nshift_col = consts[:, 257:258]   # all -SHIFT

            parts = keep.tile([P, 4 * CO], F32R, tag="parts")
            s_part = parts[:, 0:CO]
            t_part = parts[:, CO:2 * CO]
            bqp = parts[:, 2 * CO:3 * CO]
            bkp = parts[:, 3 * CO:4 * CO]
            tp16 = keep.tile([P, CO], F16, tag="tp16")
            bv_eff = keep.tile([1, C], F32R, tag="bv_eff")

            with (
                tc.tile_pool(name="xpool", bufs=1) as xpool,
                tc.tile_pool(name="w16p", bufs=1) as w16p,
                tc.tile_pool(name="w8p", bufs=1) as w8p,
                tc.tile_pool(name="sqp", bufs=2) as sqp,
                tc.tile_pool(name="spool", bufs=1) as spool,
                tc.tile_pool(name="prows", bufs=1) as prows,
                tc.tile_pool(name="stats_ps", bufs=1, space="PSUM") as stats_ps,
            ):
                # ---- input DMAs ----
                xT8 = xpool.tile([P, CO, N], F8, tag="xT8", name="xT8")
                for o in range(CO):
                    eng = nc.sync if o % 2 == 0 else nc.gpsimd
                    eng.dma_start(xT8[:, o, :], xT8_d[o * P:(o + 1) * P, :])

                w16 = {}
                for name, src_d in (("wq", wq_d), ("wk", wk_d), ("wv", wv_d)):
                    w = w16p.tile([P, CO, C], F16, tag=name, name=name)
                    for o in range(CO):
                        nc.sync.dma_start(w[:, o, :], src_d[o * P:(o + 1) * P, :])
                    w16[name] = w

                irows = prows.tile([1, 5 * C], F32R, tag="irows")
                gamma_row = irows[:, 0 * C:1 * C]
                beta_row = irows[:, 1 * C:2 * C]
                bq_row = irows[:, 2 * C:3 * C]
                bk_row = irows[:, 3 * C:4 * C]
                bv_row = irows[:, 4 * C:5 * C]
                for i, src_d in enumerate((gamma_d, beta_d, bq_d, bk_d, bv_d)):
                    nc.sync.dma_start(irows[:, i * C:(i + 1) * C],
                                      src_d[:][None, :])

                wrows = prows.tile([1, 4 * C], F32, tag="wrows")
                sum_row = wrows[:, 0 * C:1 * C]
                sq_row = wrows[:, 1 * C:2 * C]
                s_row = wrows[:, 2 * C:3 * C].bitcast(F32R)
                t_row = wrows[:, 3 * C:4 * C].bitcast(F32R)
                berows = prows.tile([1, 2 * C], F32R, tag="berows")
                grows = prows.tile([1, 3 * G], F32, tag="grows")
                g_mean = grows[:, 0:G]
                g_var = grows[:, G:2 * G]
                g_tmp = grows[:, 2 * G:3 * G]

                # ---- group-norm stats from fp8 xT ----
                # statblk cols 0:4 = channel sums (per o), 4:8 = sumsq
                statblk = spool.tile([P, 8], F32, tag="statblk")
                for o in range(CO):
                    nc.vector.reduce_sum(statblk[:, o:o + 1], xT8[:, o, :],
                                         axis=mybir.AxisListType.X)
                # sum of squares on ScalarE (Square + accum_out; the DVE
                # tensor_tensor_reduce accum path crashes on hardware)
                for o in range(CO):
                    sqd = sqp.tile([P, N], F8, tag="sqd", name="sqd")
                    nc.scalar.activation(sqd[:], xT8[:, o, :], AF.Square,
                                         accum_out=statblk[:, 4 + o:5 + o])

                # transpose each [128,1] stat column into row layout
                sums_ps = stats_ps.tile([1, C], F32, tag="sums", name="sums")
                sqs_ps = stats_ps.tile([1, C], F32, tag="sqs", name="sqs")
                for o in range(CO):
                    nc.tensor.matmul(sums_ps[0:1, o * P:(o + 1) * P],
                                     statblk[:, o:o + 1], _f(ident),
                                     is_transpose=True,
                                     start=(o == 0), stop=(o == CO - 1))
                for o in range(CO):
                    nc.tensor.matmul(sqs_ps[0:1, o * P:(o + 1) * P],
                                     statblk[:, 4 + o:5 + o], _f(ident),
                                     is_transpose=True,
                                     start=(o == 0), stop=(o == CO - 1))
                nc.vector.tensor_copy(sum_row, sums_ps[:])
                nc.vector.tensor_copy(sq_row, sqs_ps[:])

                # ---- group stats -> per-channel scale/shift ----
                inv_cnt = 1.0 / (N * GS)
                nc.vector.reduce_sum(g_mean,
                                     sum_row.rearrange("p (g e) -> p g e", e=GS),
                                     axis=mybir.AxisListType.X)
                nc.vector.tensor_scalar_mul(g_mean, g_mean, inv_cnt)
                nc.vector.reduce_sum(g_var,
                                     sq_row.rearrange("p (g e) -> p g e", e=GS),
                                     axis=mybir.AxisListType.X)
                nc.vector.tensor_scalar_mul(g_var, g_var, inv_cnt)
                nc.vector.tensor_mul(g_tmp, g_mean, g_mean)
                nc.vector.tensor_sub(g_var, g_var, g_tmp)
                nc.vector.tensor_scalar_add(g_var, g_var, EPS)
                nc.scalar.activation(g_tmp, g_var, AF.Sqrt)
                nc.vector.reciprocal(g_tmp, g_tmp)  # rstd per group

                sv = s_row.rearrange("p (g e) -> p g e", e=GS)
                tv = t_row.rearrange("p (g e) -> p g e", e=GS)
                gv = gamma_row.rearrange("p (g e) -> p g e", e=GS)
                nc.vector.tensor_tensor(
                    sv, gv, g_tmp[:, :, None].to_broadcast((1, G, GS)),
                    ALU.mult)
                nc.vector.tensor_tensor(
                    tv, sv, g_mean[:, :, None].to_broadcast((1, G, GS)),
                    ALU.mult)
                nc.vector.tensor_sub(t_row, beta_row, t_row)

                with tc.tile_pool(name="pize_ps", bufs=1, space="PSUM") as pize_ps:
                    for vec_row, dst in ((s_row, s_part), (t_row, t_part)):
                        pp = pize_ps.tile([P, CO], F32, tag="pize", name="pp")
                        for o in range(CO):
                            nc.tensor.matmul(pp[:, o:o + 1],
                                             _f(vec_row[0:1, o * P:(o + 1) * P]),
                                             _f(ones_row[0:1, 0:1]),
                                             start=(o == 0), stop=(o == CO - 1))
                        nc.vector.tensor_copy(dst, pp[:])
                    nc.vector.tensor_copy(tp16[:], t_part)

                    # effective biases b' = t @ W + b  (w16 holds 16*w)
                    beff = {"wq": berows[:, 0:C], "wk": berows[:, C:2 * C],
                            "wv": bv_eff[:]}
                    for name, brow in (("wq", bq_row), ("wk", bk_row),
                                       ("wv", bv_row)):
                        bps = stats_ps.tile([1, C], F32, tag="bps", name="bps")
                        for o in range(CO):
                            nc.tensor.matmul(bps[:], tp16[:, o:o + 1],
                                             w16[name][:, o, :],
                                             start=(o == 0), stop=(o == CO - 1))
                        nc.vector.scalar_tensor_tensor(
                            beff[name], bps[:], 1.0 / WSCALE, brow,
                            ALU.mult, ALU.add)

                    for vec_row, dst in ((beff["wq"], bqp), (beff["wk"], bkp)):
                        pp = pize_ps.tile([P, CO], F32, tag="pize", name="pp")
                        for o in range(CO):
                            nc.tensor.matmul(pp[:, o:o + 1],
                                             _f(vec_row[0:1, o * P:(o + 1) * P]),
                                             _f(ones_row[0:1, 0:1]),
                                             start=(o == 0), stop=(o == CO - 1))
                        nc.vector.tensor_copy(dst, pp[:])

                # fold group-norm scale into fp8 copies of 16*wq/wk/wv
                w8 = {}
                for name in ("wq", "wk", "wv"):
                    w = w8p.tile([P, CO, C], F8, tag=name, name=f"{name}8")
                    for o in range(CO):
                        nc.vector.tensor_scalar_mul(w[:, o, :],
                                                    w16[name][:, o, :],
                                                    _f(s_part[:, o:o + 1]))
                    w8[name] = w

                # ---- projections (fp8 DoubleRow; psum holds 16x values) ----
                def stage_out(dst, ps, bias_part, idx):
                    """psum/16 + bias -> fp8, alternating ScalarE / DVE."""
                    if idx % 2 == 0:
                        nc.scalar.activation(dst, ps, AF.Identity,
                                             bias=_f(bias_part),
                                             scale=1.0 / WSCALE)
                    else:
                        nc.vector.scalar_tensor_tensor(
                            dst, ps, 1.0 / WSCALE,
                            _f(bias_part).to_broadcast((P, ps.shape[-1])),
                            ALU.mult, ALU.add)

                with tc.tile_pool(name="proj_ps", bufs=1,
                                  space="PSUM") as proj_ps:
                    # K: all 4096 keys
                    for o in range(CO):
                        for jcb in range(2):
                            kpss = [proj_ps.tile([P, 512], F32,
                                                 tag=f"proj{jc}",
                                                 name=f"kps{jc}")
                                    for jc in range(4)]
                            for cp in range(CPAIR):
                                for jc in range(4):
                                    col = (jcb * 4 + jc) * 512
                                    nc.tensor.matmul(
                                        kpss[jc][:],
                                        w8["wk"][:, 2 * cp:2 * cp + 2,
                                                 o * P:(o + 1) * P],
                                        xT8[:, 2 * cp:2 * cp + 2,
                                            col:col + 512],
                                        start=(cp == 0),
                                        stop=(cp == CPAIR - 1),
                                        perf_mode=DR)
                            for jc in range(4):
                                col = (jcb * 4 + jc) * 512
                                stage_out(kT8[:, o, col:col + 512],
                                          kpss[jc][:], bkp[:, o:o + 1], jc)

                    # Q: own 2048 queries
                    for o in range(CO):
                        qpss = [proj_ps.tile([P, 512], F32, tag=f"proj{jc}",
                                             name=f"qps{jc}")
                                for jc in range(4)]
                        for cp in range(CPAIR):
                            for jc in range(4):
                                nc.tensor.matmul(
                                    qpss[jc][:],
                                    w8["wq"][:, 2 * cp:2 * cp + 2,
                                             o * P:(o + 1) * P],
                                    xT8[:, 2 * cp:2 * cp + 2,
                                        jc * 512:(jc + 1) * 512],
                                    start=(cp == 0), stop=(cp == CPAIR - 1),
                                    perf_mode=DR)
                        for jc in range(4):
                            stage_out(qT8[:, o, jc * 512:(jc + 1) * 512],
                                      qpss[jc][:], bqp[:, o:o + 1], jc + 1)

                    # V rows (bias folded in later via denom outer-product)
                    for t16 in range(NT):
                        vps = proj_ps.tile([P, C], F32, tag=f"proj{t16 % 4}",
                                           name="vps")
                        for cp in range(CPAIR):
                            nc.tensor.matmul(
                                vps[:],
                                xT8[:, 2 * cp:2 * cp + 2,
                                    t16 * P:(t16 + 1) * P],
                                w8["wv"][:, 2 * cp:2 * cp + 2, :],
                                start=(cp == 0), stop=(cp == CPAIR - 1),
                                perf_mode=DR)
                        if t16 % 2 == 0:
                            nc.vector.tensor_scalar_mul(v8[:, t16, :], vps[:],
                                                        1.0 / WSCALE)
                        else:
                            nc.scalar.activation(v8[:, t16, :], vps[:],
                                                 AF.Copy, scale=1.0 / WSCALE)

            # ---- attention + output projection + residual ----
            with (
                tc.tile_pool(name="wop", bufs=1) as wop,
                tc.tile_pool(name="sT_ps", bufs=2, space="PSUM") as sT_ps,
                tc.tile_pool(name="av_ps", bufs=1, space="PSUM") as av_ps,
                tc.tile_pool(name="sh_ps", bufs=2, space="PSUM") as sh_ps,
                tc.tile_pool(name="expp", bufs=4) as expp,
                tc.tile_pool(name="accp", bufs=2) as accp,
                tc.tile_pool(name="aoT", bufs=2) as aoTp,
                tc.tile_pool(name="ostage", bufs=2) as ostage,
                tc.tile_pool(name="xres", bufs=2) as xres,
                tc.tile_pool(name="drow", bufs=2) as drow,
            ):
                wo16 = wop.tile([P, CO, C], F16, tag="wo", name="wo16")
                for o in range(CO):
                    nc.sync.dma_start(wo16[:, o, :], wo_d[o * P:(o + 1) * P, :])

                for chunk in range(N_CHUNKS):
                    i0 = chunk * I_CHUNK
                    avs = [av_ps.tile([P, I_CHUNK], F32, tag=f"av{i}",
                                      name=f"av{i}")
                           for i in range(CO)]
                    acc_a = accp.tile([P, I_CHUNK], F32, tag="acc_a")
                    acc_b = accp.tile([P, I_CHUNK], F32, tag="acc_b")
                    for jp in range(JPAIR):
                        ex = expp.tile([P, 2, I_CHUNK], F8, tag="ex")
                        for jj in range(2):
                            j = 2 * jp + jj
                            sps = sT_ps.tile([P, I_CHUNK], F32, tag="sT",
                                             name="sps")
                            for cp in range(CPAIR):
                                nc.tensor.matmul(
                                    sps[:],
                                    kT8[:, 2 * cp:2 * cp + 2,
                                        j * P:(j + 1) * P],
                                    qT8[:, 2 * cp:2 * cp + 2,
                                        i0:i0 + I_CHUNK],
                                    start=(cp == 0), stop=(cp == CPAIR - 1),
                                    perf_mode=DR)
                            nc.scalar.activation(ex[:, jj, :], sps[:], AF.Exp,
                                                 scale=SM_SCALE,
                                                 bias=_f(nshift_col))
                            # denominator partials (fp8 adds on DVE)
                            if j == 0:
                                nc.vector.tensor_copy(acc_a[:], ex[:, jj, :])
                            elif j == 1:
                                nc.vector.tensor_copy(acc_b[:], ex[:, jj, :])
                            elif j % 2 == 0:
                                nc.vector.tensor_add(acc_a[:], acc_a[:],
                                                     ex[:, jj, :])
                            else:
                                nc.vector.tensor_add(acc_b[:], acc_b[:],
                                                     ex[:, jj, :])
                        for cs in range(CO):
                            nc.tensor.matmul(avs[cs][:],
                                             v8[:, 2 * jp:2 * jp + 2,
                                                cs * P:(cs + 1) * P],
                                             ex[:, :, :],
                                             start=(jp == 0), stop=False,
                                             perf_mode=DR)

                    nc.vector.tensor_add(acc_a[:], acc_a[:], acc_b[:])
                    dps = sh_ps.tile([1, I_CHUNK], F32, tag="sh", name="dps")
                    nc.tensor.matmul(dps[:], _f(ones_col), _f(acc_a[:]),
                                     start=True, stop=True)
                    d_row = drow.tile([1, I_CHUNK], F32R, tag="d_row")
                    nc.vector.tensor_copy(d_row[:], dps[:])
                    # V-bias: avT += bv (x) denom (unnormalized rows sum to denom)
                    for cs in range(CO):
                        nc.tensor.matmul(avs[cs][:],
                                         bv_eff[0:1, cs * P:(cs + 1) * P],
                                         d_row[:],
                                         start=False, stop=True)
                    dp = sh_ps.tile([P, 4], F32, tag="sh", name="dp")
                    for o in range(4):
                        nc.tensor.matmul(dp[:, o:o + 1],
                                         _f(d_row[0:1, o * P:(o + 1) * P]),
                                         _f(ones_row[0:1, 0:1]),
                                         start=(o == 0), stop=(o == 3))
                    d_inv = drow.tile([P, 4], F32, tag="d_inv")
                    nc.vector.reciprocal(d_inv[:], dp[:])

                    aoT = aoTp.tile([P, CO, I_CHUNK], F16, tag="aoT")
                    for cs in range(CO):
                        if cs % 2 == 0:
                            nc.vector.tensor_copy(aoT[:, cs, :], avs[cs][:])
                        else:
                            nc.scalar.activation(aoT[:, cs, :], avs[cs][:],
                                                 AF.Copy)

                    for it in range(4):
                        ops = sh_ps.tile([P, C], F32, tag="sh", name="ops")
                        for ci in range(CO):
                            nc.tensor.matmul(ops[:],
                                             aoT[:, ci, it * P:(it + 1) * P],
                                             wo16[:, ci, :],
                                             start=(ci == 0),
                                             stop=(ci == CO - 1))
                        xr = xres.tile([P, C], F32R, tag="xr")
                        nc.sync.dma_start(xr[:], xbo_t[chunk * 4 + it])
                        ot = ostage.tile([P, C], F32, tag="ot")
                        nc.vector.scalar_tensor_tensor(
                            ot[:], ops[:], _f(d_inv[:, it:it + 1]), xr[:],
                            ALU.mult, ALU.add)
                        nc.sync.dma_start(out_t[chunk * 4 + it], ot[:])

    nc.compile()
    return nc


_NC = None


def _get_nc():
    global _NC
    if _NC is None:
        _NC = build_nc()
    return _NC


def make_consts():
    consts = np.zeros((P, 900), np.float32)
    consts[:, 0:P] = np.eye(P, dtype=np.float32)
    consts[:, P] = 1.0
    consts[0, 129:257] = 1.0
    consts[:, 257] = -SHIFT
    return consts


def make_in_maps(x, gn_gamma, gn_beta, wq, bq, wk, bk, wv, bv, wo, bo):
    x4 = np.ascontiguousarray(np.asarray(x, np.float32).reshape(B, N, C))
    consts = make_consts()
    bo_f = np.asarray(bo, np.float32)
    common = dict(
        wq16=(WSCALE * np.asarray(wq, np.float32)).astype(np.float16),
        wk16=(WSCALE * np.asarray(wk, np.float32)).astype(np.float16),
        wv16=(WSCALE * np.asarray(wv, np.float32)).astype(np.float16),
        wo16=np.asarray(wo, np.float32).astype(np.float16),
        bq=np.asarray(bq, np.float32), bk=np.asarray(bk, np.float32),
        bv=np.asarray(bv, np.float32),
        gn_gamma=np.asarray(gn_gamma, np.float32),
        gn_beta=np.asarray(gn_beta, np.float32),
        consts=consts,
    )
    in_maps = []
    for c in range(N_CORES):
        b, h = c // 2, c % 2
        own = x4[b, h * HALF:(h + 1) * HALF]
        other = x4[b, (1 - h) * HALF:(2 - h) * HALF]
        xb_ = np.concatenate([own, other], axis=0)          # [N, C]
        xT8 = np.ascontiguousarray(xb_.T).astype(ml_dtypes.float8_e4m3)
        xbo = np.ascontiguousarray(own + bo_f)
        in_maps.append(dict(xT8=xT8, xbo=xbo, **common))
    return in_maps


def assemble(results):
    out = np.empty((B, N, C), np.float32)
    for c in range(N_CORES):
        b, h = c // 2, c % 2
        out[b, h * HALF:(h + 1) * HALF] = results[c]["out"]
    return out.reshape(B, 64, 64, C)


def kernel(**inputs):
    nc = _get_nc()
    in_maps = make_in_maps(**inputs)
    res = run_bass_kernel_spmd(nc, in_maps, list(range(N_CORES)))
    return assemble(res.results)
